# revision 16
# baseline (speedup 1.0000x reference)
"""ALiBi bias subtraction on Trainium2, SPMD across 8 NeuronCores.

out[b,h,i,j] = scores[b,h,i,j] - slope_h * (i - j)

(The `offset` input cancels in pos_diff = (i+off) - (j+off), so it never
enters the computation.)

Sharding: flatten (B=2, H=16) -> 32 slices of [2048, 2048]; core c takes
slices [4c, 4c+4). The bias is built locally per core:
  out = (scores + negrow) + colb
    negrow[p]  = -slope * (row index)     (per-partition bias, ScalarEngine)
    colb[p,j]  = +slope * j               (tensor_tensor add, VectorEngine)
Both are generated on-device from iota + per-core slope vectors, so the
only meaningful HBM traffic is scores in + out out (memory roofline).
"""

import sys

if "/opt/trn_rl_repo" not in sys.path:
    sys.path.insert(0, "/opt/trn_rl_repo")

import numpy as np

B, H, S = 2, 16, 2048
N_CORES = 8
SPC = (B * H) // N_CORES  # 4 slices per core
P = 128                   # partitions
NB = S // P               # 16 row-blocks per slice

_NC_CACHE = {}


def _build_nc():
    import concourse.bacc as bacc
    import concourse.mybir as mybir
    from concourse.tile import TileContext

    f32 = mybir.dt.float32
    nc = bacc.Bacc()
    scores = nc.declare_dram_parameter("scores", [SPC, S, S], f32, isOutput=False)
    colb_in = nc.declare_dram_parameter("colb", [P, SPC * S], f32, isOutput=False)
    negrow_in = nc.declare_dram_parameter(
        "negrow", [P, SPC * NB], f32, isOutput=False
    )
    out = nc.declare_dram_parameter("out", [SPC, S, S], f32, isOutput=True)

    with TileContext(nc) as tc:
        with tc.tile_pool(name="const", bufs=1) as cpool:
            # colb[p, s*S + j]  = slope_s * j
            # negrow[p, s*NB+b] = -slope_s * (128*b + p)
            colb = cpool.tile([P, SPC * S], f32, tag="colb")
            negrow = cpool.tile([P, SPC * NB], f32, tag="negrow")
            nc.sync.dma_start(out=colb[:], in_=colb_in[:])
            nc.sync.dma_start(out=negrow[:], in_=negrow_in[:])

            with tc.tile_pool(name="work", bufs=10) as pool:
                for s in range(SPC):
                    for b in range(NB):
                        tile = pool.tile([P, S], f32, tag="t")
                        nc.sync.dma_start(
                            out=tile[:], in_=scores[s, b * P:(b + 1) * P, :]
                        )
                        idx = s * NB + b
                        nc.scalar.activation(
                            tile[:], tile[:],
                            mybir.ActivationFunctionType.Identity,
                            bias=negrow[:, idx:idx + 1], scale=1.0,
                        )
                        nc.vector.tensor_add(
                            out=tile[:], in0=tile[:], in1=colb[:, s * S:(s + 1) * S]
                        )
                        nc.scalar.dma_start(
                            out=out[s, b * P:(b + 1) * P, :], in_=tile[:]
                        )
    nc.compile()
    return nc


def _get_nc():
    if "nc" not in _NC_CACHE:
        _NC_CACHE["nc"] = _build_nc()
    return _NC_CACHE["nc"]


def _make_in_maps(scores_np):
    flat = np.ascontiguousarray(
        np.asarray(scores_np, dtype=np.float32).reshape(B * H, S, S)
    )
    slopes_full = (
        2.0 ** (-8.0 * np.arange(1, H + 1, dtype=np.float32) / np.float32(H))
    ).astype(np.float32)
    j_idx = np.arange(S, dtype=np.float32)           # [S]
    p_idx = np.arange(P, dtype=np.float32)           # [P]
    b_idx = np.arange(NB, dtype=np.float32)          # [NB]
    row_idx = P * b_idx[None, :] + p_idx[:, None]    # [P, NB] = 128*b + p
    in_maps = []
    for c in range(N_CORES):
        gs = np.arange(c * SPC, (c + 1) * SPC)
        sl = slopes_full[gs % H]  # [SPC]
        # colb[p, s, j] = slope_s * j  (replicated over partitions p)
        colb = np.broadcast_to(
            sl[None, :, None] * j_idx[None, None, :], (P, SPC, S)
        ).reshape(P, SPC * S)
        # negrow[p, s, b] = -slope_s * (128*b + p)
        negrow = (-sl[None, :, None] * row_idx[:, None, :]).reshape(P, SPC * NB)
        in_maps.append({
            "scores": np.ascontiguousarray(flat[c * SPC:(c + 1) * SPC]),
            "colb": np.ascontiguousarray(colb.astype(np.float32)),
            "negrow": np.ascontiguousarray(negrow.astype(np.float32)),
        })
    return in_maps


def run(scores, offset=0, trace=False, **trace_kwargs):
    """Returns (full_output, BassKernelResults)."""
    from concourse.bass_utils import run_bass_kernel_spmd

    nc = _get_nc()
    in_maps = _make_in_maps(scores)
    res = run_bass_kernel_spmd(
        nc, in_maps, core_ids=list(range(N_CORES)), trace=trace, **trace_kwargs
    )
    outs = [np.asarray(res.results[c]["out"]) for c in range(N_CORES)]
    full = np.concatenate(outs, axis=0).reshape(B, H, S, S)
    return full, res


def kernel(scores, offset=0):
    full, _ = run(scores, offset, trace=False)
    return full


# revision 20
# speedup vs baseline: 1.0874x; 1.0874x over previous
"""ALiBi bias subtraction on Trainium2, SPMD across 8 NeuronCores.

out[b,h,i,j] = scores[b,h,i,j] - slope_h * (i - j)

(The `offset` input cancels in pos_diff = (i+off) - (j+off), so it never
enters the computation.)

Sharding: flatten (B=2, H=16) -> 32 slices of [2048, 2048]; core c takes
slices [4c, 4c+4). The bias is built locally per core:
  out = (scores + negrow) + colb
    negrow[p]  = -slope * (row index)     (per-partition bias, ScalarEngine)
    colb[p,j]  = +slope * j               (tensor_tensor add, VectorEngine)
Both are generated on-device from iota + per-core slope vectors, so the
only meaningful HBM traffic is scores in + out out (memory roofline).
"""

import sys

if "/opt/trn_rl_repo" not in sys.path:
    sys.path.insert(0, "/opt/trn_rl_repo")

import numpy as np

B, H, S = 2, 16, 2048
N_CORES = 8
SPC = (B * H) // N_CORES  # 4 slices per core
P = 128                   # partitions
NB = S // P               # 16 row-blocks per slice

_NC_CACHE = {}


def _build_nc(bufs=10, split_rings=True, nbb=1):
    import concourse.bacc as bacc
    import concourse.mybir as mybir
    from concourse.tile import TileContext

    f32 = mybir.dt.float32
    nc = bacc.Bacc()
    scores = nc.declare_dram_parameter("scores", [SPC, S, S], f32, isOutput=False)
    slopes_in = nc.declare_dram_parameter("slopes", [P, SPC], f32, isOutput=False)
    negrow_in = nc.declare_dram_parameter(
        "negrow", [P, SPC * NB], f32, isOutput=False
    )
    out = nc.declare_dram_parameter("out", [SPC, S, S], f32, isOutput=True)

    with TileContext(nc) as tc:
        with tc.tile_pool(name="const", bufs=1) as cpool:
            # colb[p, s*S + j]  = slope_s * j      (device-built from iota;
            #   J is exact for 0 <= j < 2^24 in f32, and J*slope rounds the
            #   same way the host-side slope_s*j would)
            # negrow[p, s*NB+b] = -slope_s * (128*b + p)   (host-built, 32KB)
            colb = cpool.tile([P, SPC * S], f32, tag="colb")
            negrow = cpool.tile([P, SPC * NB], f32, tag="negrow")
            slopes_t = cpool.tile([P, SPC], f32, tag="slopes_t")
            nc.sync.dma_start(out=slopes_t[:], in_=slopes_in[:])
            J = cpool.tile([P, S], f32, tag="J")
            nc.gpsimd.iota(
                J[:], [[1, S]], channel_multiplier=0,
                allow_small_or_imprecise_dtypes=True,
            )
            for s in range(SPC):
                nc.vector.tensor_scalar_mul(
                    colb[:, s * S:(s + 1) * S], J[:], slopes_t[:, s:s + 1]
                )
            nc.sync.dma_start(out=negrow[:], in_=negrow_in[:])

            with tc.tile_pool(name="work", bufs=bufs) as pool:
                for s in range(SPC):
                    sc_r = scores[s].rearrange("(a p) j -> p a j", p=P)
                    out_r = out[s].rearrange("(a p) j -> p a j", p=P)
                    for bb in range(NB // nbb):
                        tile = pool.tile([P, nbb, S], f32, tag="t")
                        nc.sync.dma_start(
                            out=tile[:],
                            in_=sc_r[:, bb * nbb:(bb + 1) * nbb, :],
                        )
                        for c in range(nbb):
                            idx = s * NB + bb * nbb + c
                            nc.scalar.activation(
                                tile[:, c, :], tile[:, c, :],
                                mybir.ActivationFunctionType.Identity,
                                bias=negrow[:, idx:idx + 1], scale=1.0,
                            )
                            nc.vector.tensor_add(
                                out=tile[:, c, :], in0=tile[:, c, :],
                                in1=colb[:, s * S:(s + 1) * S],
                            )
                        out_eng = nc.scalar if split_rings else nc.sync
                        out_eng.dma_start(
                            out=out_r[:, bb * nbb:(bb + 1) * nbb, :], in_=tile[:]
                        )
    nc.compile()
    return nc


def _get_nc():
    if "nc" not in _NC_CACHE:
        _NC_CACHE["nc"] = _build_nc()
    return _NC_CACHE["nc"]


def _make_in_maps(scores_np):
    flat = np.ascontiguousarray(
        np.asarray(scores_np, dtype=np.float32).reshape(B * H, S, S)
    )
    slopes_full = (
        2.0 ** (-8.0 * np.arange(1, H + 1, dtype=np.float32) / np.float32(H))
    ).astype(np.float32)
    j_idx = np.arange(S, dtype=np.float32)           # [S]
    p_idx = np.arange(P, dtype=np.float32)           # [P]
    b_idx = np.arange(NB, dtype=np.float32)          # [NB]
    row_idx = P * b_idx[None, :] + p_idx[:, None]    # [P, NB] = 128*b + p
    in_maps = []
    for c in range(N_CORES):
        gs = np.arange(c * SPC, (c + 1) * SPC)
        sl = slopes_full[gs % H]  # [SPC]
        # negrow[p, s, b] = -slope_s * (128*b + p)
        negrow = (-sl[None, :, None] * row_idx[:, None, :]).reshape(P, SPC * NB)
        in_maps.append({
            "scores": np.ascontiguousarray(flat[c * SPC:(c + 1) * SPC]),
            "slopes": np.ascontiguousarray(
                np.broadcast_to(sl, (P, SPC)).astype(np.float32)
            ),
            "negrow": np.ascontiguousarray(negrow.astype(np.float32)),
        })
    return in_maps


def run(scores, offset=0, trace=False, **trace_kwargs):
    """Returns (full_output, BassKernelResults)."""
    from concourse.bass_utils import run_bass_kernel_spmd

    nc = _get_nc()
    in_maps = _make_in_maps(scores)
    res = run_bass_kernel_spmd(
        nc, in_maps, core_ids=list(range(N_CORES)), trace=trace, **trace_kwargs
    )
    outs = [np.asarray(res.results[c]["out"]) for c in range(N_CORES)]
    full = np.concatenate(outs, axis=0).reshape(B, H, S, S)
    return full, res


def kernel(scores, offset=0):
    full, _ = run(scores, offset, trace=False)
    return full


# revision 25
# speedup vs baseline: 1.1232x; 1.0330x over previous
"""ALiBi bias subtraction on Trainium2, SPMD across 8 NeuronCores.

out[b,h,i,j] = scores[b,h,i,j] - slope_h * (i - j)

(The `offset` input cancels in pos_diff = (i+off) - (j+off), so it never
enters the computation.)

Sharding: flatten (B=2, H=16) -> 32 slices of [2048, 2048]; core c takes
slices [4c, 4c+4). The bias is applied locally per core in two engine-
parallel elementwise passes over each [128, 2048] tile:
  out = (scores + negrow) + colb
    negrow[p]  = -slope * (row index)  - per-partition bias on ScalarEngine
                 (host-precomputed, 32KB input)
    colb[p,j]  = +slope * j            - tensor_tensor add on VectorEngine
                 (device-built: gpsimd iota J[p,j]=j, then J * slope_s with
                  slope_s a per-core [128,1] scalar input; saves 4MiB of
                  HBM traffic vs DMAing the replicated table)
Loads ride the sync-engine HWDGE ring, stores the scalar-engine ring
(doubling descriptor-gen throughput); bufs=10 slots keep 16 SDMA engines
~95%+ busy. Measured fast-mode exec ~331us vs ~317us pure-DMA floor
(128 MiB/core at ~425 GB/s effective) + ~6us ramp + ~9us Tile epilogue.
"""

import sys

if "/opt/trn_rl_repo" not in sys.path:
    sys.path.insert(0, "/opt/trn_rl_repo")

import numpy as np

B, H, S = 2, 16, 2048
N_CORES = 8
SPC = (B * H) // N_CORES  # 4 slices per core
P = 128                   # partitions
NB = S // P               # 16 row-blocks per slice

_NC_CACHE = {}


def _build_nc(bufs=10, split_rings=True, nbb=1):
    import concourse.bacc as bacc
    import concourse.mybir as mybir
    from concourse.tile import TileContext

    f32 = mybir.dt.float32
    nc = bacc.Bacc()
    scores = nc.declare_dram_parameter("scores", [SPC, S, S], f32, isOutput=False)
    slopes_in = nc.declare_dram_parameter("slopes", [P, SPC], f32, isOutput=False)
    negrow_in = nc.declare_dram_parameter(
        "negrow", [P, SPC * NB], f32, isOutput=False
    )
    out = nc.declare_dram_parameter("out", [SPC, S, S], f32, isOutput=True)

    with TileContext(nc) as tc:
        with tc.tile_pool(name="const", bufs=1) as cpool:
            # colb[p, s*S + j]  = slope_s * j      (device-built from iota;
            #   J is exact for 0 <= j < 2^24 in f32, and J*slope rounds the
            #   same way the host-side slope_s*j would)
            # negrow[p, s*NB+b] = -slope_s * (128*b + p)   (host-built, 32KB)
            colb = cpool.tile([P, SPC * S], f32, tag="colb")
            negrow = cpool.tile([P, SPC * NB], f32, tag="negrow")
            slopes_t = cpool.tile([P, SPC], f32, tag="slopes_t")
            nc.sync.dma_start(out=slopes_t[:], in_=slopes_in[:])
            J = cpool.tile([P, S], f32, tag="J")
            nc.gpsimd.iota(
                J[:], [[1, S]], channel_multiplier=0,
                allow_small_or_imprecise_dtypes=True,
            )
            for s in range(SPC):
                nc.vector.tensor_scalar_mul(
                    colb[:, s * S:(s + 1) * S], J[:], slopes_t[:, s:s + 1]
                )
            nc.sync.dma_start(out=negrow[:], in_=negrow_in[:])

            with tc.tile_pool(name="work", bufs=bufs) as pool:
                for s in range(SPC):
                    sc_r = scores[s].rearrange("(a p) j -> p a j", p=P)
                    out_r = out[s].rearrange("(a p) j -> p a j", p=P)
                    for bb in range(NB // nbb):
                        tile = pool.tile([P, nbb, S], f32, tag="t")
                        nc.sync.dma_start(
                            out=tile[:],
                            in_=sc_r[:, bb * nbb:(bb + 1) * nbb, :],
                        )
                        for c in range(nbb):
                            idx = s * NB + bb * nbb + c
                            nc.scalar.activation(
                                tile[:, c, :], tile[:, c, :],
                                mybir.ActivationFunctionType.Identity,
                                bias=negrow[:, idx:idx + 1], scale=1.0,
                            )
                            nc.vector.tensor_add(
                                out=tile[:, c, :], in0=tile[:, c, :],
                                in1=colb[:, s * S:(s + 1) * S],
                            )
                        out_eng = nc.scalar if split_rings else nc.sync
                        out_eng.dma_start(
                            out=out_r[:, bb * nbb:(bb + 1) * nbb, :], in_=tile[:]
                        )
    nc.compile()
    return nc


def _build_nc_raw(bufs=10, lag=3):
    """Hand-scheduled raw-Bass variant: same dataflow as _build_nc but with
    explicit per-engine instruction streams and semaphores, and a minimal
    epilogue (single final wait + sem clear) instead of Tile's
    drain + double all-engine barrier (~9us tail)."""
    import concourse.bacc as bacc
    import concourse.mybir as mybir

    f32 = mybir.dt.float32
    NT = SPC * NB  # 64 tiles
    nc = bacc.Bacc()
    scores = nc.declare_dram_parameter("scores", [SPC, S, S], f32, isOutput=False)
    slopes_in = nc.declare_dram_parameter("slopes", [P, SPC], f32, isOutput=False)
    negrow_in = nc.declare_dram_parameter(
        "negrow", [P, SPC * NB], f32, isOutput=False
    )
    out = nc.declare_dram_parameter("out", [SPC, S, S], f32, isOutput=True)

    with (
        nc.sbuf_tensor("tiles", [P, bufs, S], f32) as tiles,
        nc.sbuf_tensor("colb", [P, SPC * S], f32) as colb,
        nc.sbuf_tensor("negrow_sb", [P, SPC * NB], f32) as negrow,
        nc.sbuf_tensor("slopes_t", [P, SPC], f32) as slopes_t,
        nc.sbuf_tensor("J", [P, S], f32) as J,
        nc.semaphore("s_in") as s_in,
        nc.semaphore("s_act") as s_act,
        nc.semaphore("s_tt") as s_tt,
        nc.semaphore("s_out") as s_out,
        nc.semaphore("s_iota") as s_iota,
        nc.Block() as block,
    ):
        sems = [s_in, s_act, s_tt, s_out, s_iota]

        @block.sync
        def _(sync):
            sync.dma_start(out=slopes_t[:], in_=slopes_in[:]).then_inc(s_in, 16)
            sync.dma_start(out=negrow[:], in_=negrow_in[:]).then_inc(s_in, 16)
            for k in range(NT):
                s, b = divmod(k, NB)
                if k >= bufs:
                    sync.wait_ge(s_out, 16 * (k - bufs + 1))
                sync.dma_start(
                    out=tiles[:, k % bufs, :],
                    in_=scores[s, b * P:(b + 1) * P, :],
                ).then_inc(s_in, 16)


        @block.gpsimd
        def _(gpsimd):
            gpsimd.iota(
                J[:], [[1, S]], channel_multiplier=0,
                allow_small_or_imprecise_dtypes=True,
            ).then_inc(s_iota, 1)
            # epilogue: everything is transitively done once the last
            # out-DMA lands; clear sems so the NEFF can re-execute.
            gpsimd.wait_ge(s_out, 16 * NT)
            nums = sorted(sh.num for sh in sems)
            assert nums == list(range(nums[0], nums[0] + len(nums))), nums
            gpsimd.sem_clear(range(nums[0], nums[-1] + 1))

        @block.vector
        def _(vector):
            vector.wait_ge(s_iota, 1)
            vector.wait_ge(s_in, 16)  # slopes loaded (first sync DMA)
            for s in range(SPC):
                vector.tensor_scalar_mul(
                    colb[:, s * S:(s + 1) * S], J[:], slopes_t[:, s:s + 1]
                )
            for k in range(NT):
                s, b = divmod(k, NB)
                vector.wait_ge(s_act, k + 1)
                vector.tensor_add(
                    out=tiles[:, k % bufs, :],
                    in0=tiles[:, k % bufs, :],
                    in1=colb[:, s * S:(s + 1) * S],
                ).then_inc(s_tt, 1)

        @block.scalar
        def _(scalar):
            def emit_out(j):
                s2, b2 = divmod(j, NB)
                scalar.wait_ge(s_tt, j + 1)
                scalar.dma_start(
                    out=out[s2, b2 * P:(b2 + 1) * P, :],
                    in_=tiles[:, j % bufs, :],
                ).then_inc(s_out, 16)

            for k in range(NT):
                s, b = divmod(k, NB)
                idx = s * NB + b
                scalar.wait_ge(s_in, 16 * (k + 3))
                scalar.activation(
                    tiles[:, k % bufs, :], tiles[:, k % bufs, :],
                    mybir.ActivationFunctionType.Identity,
                    bias=negrow[:, idx:idx + 1], scale=1.0,
                ).then_inc(s_act, 1)
                if k >= lag:
                    emit_out(k - lag)
            for j in range(NT - lag, NT):
                emit_out(j)

    nc.compile()
    return nc


def _get_nc():
    if "nc" not in _NC_CACHE:
        _NC_CACHE["nc"] = _build_nc()
    return _NC_CACHE["nc"]


def _make_in_maps(scores_np):
    flat = np.ascontiguousarray(
        np.asarray(scores_np, dtype=np.float32).reshape(B * H, S, S)
    )
    slopes_full = (
        2.0 ** (-8.0 * np.arange(1, H + 1, dtype=np.float32) / np.float32(H))
    ).astype(np.float32)
    j_idx = np.arange(S, dtype=np.float32)           # [S]
    p_idx = np.arange(P, dtype=np.float32)           # [P]
    b_idx = np.arange(NB, dtype=np.float32)          # [NB]
    row_idx = P * b_idx[None, :] + p_idx[:, None]    # [P, NB] = 128*b + p
    in_maps = []
    for c in range(N_CORES):
        gs = np.arange(c * SPC, (c + 1) * SPC)
        sl = slopes_full[gs % H]  # [SPC]
        # negrow[p, s, b] = -slope_s * (128*b + p)
        negrow = (-sl[None, :, None] * row_idx[:, None, :]).reshape(P, SPC * NB)
        in_maps.append({
            "scores": np.ascontiguousarray(flat[c * SPC:(c + 1) * SPC]),
            "slopes": np.ascontiguousarray(
                np.broadcast_to(sl, (P, SPC)).astype(np.float32)
            ),
            "negrow": np.ascontiguousarray(negrow.astype(np.float32)),
        })
    return in_maps


def run(scores, offset=0, trace=False, **trace_kwargs):
    """Returns (full_output, BassKernelResults)."""
    from concourse.bass_utils import run_bass_kernel_spmd

    nc = _get_nc()
    in_maps = _make_in_maps(scores)
    res = run_bass_kernel_spmd(
        nc, in_maps, core_ids=list(range(N_CORES)), trace=trace, **trace_kwargs
    )
    outs = [np.asarray(res.results[c]["out"]) for c in range(N_CORES)]
    full = np.concatenate(outs, axis=0).reshape(B, H, S, S)
    return full, res


def kernel(scores, offset=0):
    full, _ = run(scores, offset, trace=False)
    return full
